# revision 22
# baseline (speedup 1.0000x reference)
"""Multi-head attention TRN2 kernel (B=4, T=2048, C=1024, H=16, D=64).

Sharding: 8 cores = 4 batches x 2 head-halves. Core c handles batch c//2 and
heads (c%2)*8 .. (c%2)*8+8 (512 of the 1024 channel columns). Each core
computes a partial output projection; the host sums the two partials per
batch and adds the bp / bv rank-1 terms.

Single software-pipelined loop per core (vs the v1 three-phase layout):
  - scores are packed 2x with PE row tiling: per head K=64 contraction,
    head 0 on array rows 0:63 (tile (0,0)), head 1 on rows 64:127
    (tile (64,0)); both run concurrently, psum [128, 2, 512].
  - exp on ACT streams continuously (double-buffered score psum); this is
    the bottleneck engine (~1.15us per chunk-pair of 2*128*512 elements).
  - q/k projections of the NEXT head pair, the v projection, and the
    output projection of the PREVIOUS block are interleaved into PE slack
    between score/attention matmuls, so no separate phases remain.
  - av keeps the ones-augmented v (M=65): psum row 64 accumulates the
    softmax denominator for free; reciprocal on DVE, partition-broadcast
    on GPSIMD, normalize on DVE (all off the critical exp path).
"""

import sys
from contextlib import ExitStack

import numpy as np

sys.path.insert(0, "/opt/trn_rl_repo")

import ml_dtypes  # noqa: E402

import concourse.bass as bass  # noqa: E402
import concourse.bacc as bacc  # noqa: E402
import concourse.mybir as mybir  # noqa: E402
import concourse.tile as tile  # noqa: E402
from concourse.bass_utils import run_bass_kernel_spmd  # noqa: E402

B, T, C, H, D = 4, 2048, 1024, 16, 64
HPC = 8          # heads per core
CC = HPC * D     # per-core channel columns = 512
NCORES = 8
BF16 = mybir.dt.bfloat16
F32 = mybir.dt.float32
BLK = 512        # tq block width
NB = T // BLK    # 4 tq blocks
KC = C // 128    # 8 contraction chunks over C
MC = CC // 128   # 4 head pairs
TC = T // 128    # 16 tk chunks

_nc_cache = {}


def _build_nc():
    if "nc" in _nc_cache:
        return _nc_cache["nc"]
    nc = bacc.Bacc("TRN2", target_bir_lowering=False, debug=False)

    xT_d = nc.dram_tensor("xT", [C, T], BF16, kind="ExternalInput").ap()
    wq_d = nc.dram_tensor("wq", [C, CC], BF16, kind="ExternalInput").ap()
    wk_d = nc.dram_tensor("wk", [C, CC], BF16, kind="ExternalInput").ap()
    wv_d = nc.dram_tensor("wv", [C, CC], BF16, kind="ExternalInput").ap()
    wp_d = nc.dram_tensor("wp", [CC, C], BF16, kind="ExternalInput").ap()
    bq_d = nc.dram_tensor("bq2", [128, MC], F32, kind="ExternalInput").ap()
    bk_d = nc.dram_tensor("bk2", [128, MC], F32, kind="ExternalInput").ap()
    out_d = nc.dram_tensor("out", [T, C], F32, kind="ExternalOutput").ap()

    EXP = mybir.ActivationFunctionType.Exp

    with tile.TileContext(nc) as tc, ExitStack() as ctx:
        # persistent tensors: unique tag each, bufs=1
        p = ctx.enter_context(tc.tile_pool(name="pers", bufs=1))
        # cycling pools
        p_e = ctx.enter_context(tc.tile_pool(name="e", bufs=6))
        p_sm = ctx.enter_context(tc.tile_pool(name="sm", bufs=2))
        p_st = ctx.enter_context(tc.tile_pool(name="st", bufs=3))
        ps_s = ctx.enter_context(tc.tile_pool(name="pss", bufs=2, space="PSUM"))
        ps_y = ctx.enter_context(tc.tile_pool(name="psy", bufs=2, space="PSUM"))
        ps_po = ctx.enter_context(tc.tile_pool(name="pspo", bufs=2, space="PSUM"))

        # ---- input DMAs: one descriptor per tensor (descriptor issue on the
        # SP queue costs ~0.6us each, so batch aggressively), biases first ----
        bq2 = p.tile([128, MC], F32, tag="bq", name="bq2t")
        nc.sync.dma_start(bq2[:], bq_d[:])
        bk2 = p.tile([128, MC], F32, tag="bk", name="bk2t")
        nc.sync.dma_start(bk2[:], bk_d[:])
        xt_all = p.tile([128, KC, T], BF16, tag="xt", name="xt_all")
        nc.sync.dma_start(xt_all[:], xT_d.rearrange("(c p) t -> p c t", p=128))
        xt = [xt_all[:, k, :] for k in range(KC)]
        wq_all = p.tile([128, KC, CC], BF16, tag="wqa", name="wq_all")
        nc.sync.dma_start(wq_all[:], wq_d.rearrange("(c p) n -> p c n", p=128))
        wq_s = [wq_all[:, k, :] for k in range(KC)]
        # wk/wv/wp tiles are declared here but their DMAs are emitted
        # just-in-time between prologue groups: a consumer's semaphore wait
        # covers every DMA queued before it was emitted, so late-emitted DMAs
        # keep the early projection matmuls off the full-input-stream gate.
        wk_all = p.tile([128, KC, CC], BF16, tag="wka", name="wk_all")
        wk_s = [wk_all[:, k, :] for k in range(KC)]
        wv_all = p.tile([128, KC, CC], BF16, tag="wva", name="wv_all")
        wv_s = [wv_all[:, k, :] for k in range(KC)]
        wp_all = p.tile([128, MC, C], BF16, tag="wpa", name="wp_all")
        wp_s = [wp_all[:, m, :] for m in range(MC)]

        # ---- PE warm-up during the DMA wait: ~13us of throwaway matmuls so
        # the HAM clock gate is at 8/8 when the real prologue starts ----
        WARMUP = False
        if WARMUP:
            wrm = p.tile([128, BLK], BF16, tag="wrm", name="wrm")
            nc.gpsimd.memset(wrm[:], 0.0)
            for i in range(30):
                wacc = ps_po.tile([1, BLK], F32, tag="pacc", name=f"wacc{i}")
                nc.tensor.matmul(wacc[:], wrm[:, 0:1], wrm[:],
                                 start=True, stop=True)

        # ---- persistent compute tensors ----
        qt = [p.tile([128, T], BF16, tag=f"qt{m}", name=f"qt{m}")
              for m in range(MC)]
        kt = [p.tile([128, T], BF16, tag=f"kt{m}", name=f"kt{m}")
              for m in range(MC)]
        vaug = [p.tile([128, HPC, D + 1], BF16, tag=f"va{t}", name=f"va{t}")
                for t in range(TC)]
        ytn = [p.tile([128, T], BF16, tag=f"ytn{m}", name=f"ytn{m}")
               for m in range(MC)]

        # ---- filler work units (each ~8 matmuls on PE + a DVE evac) ----
        def mk_v(t):
            def f():
                va = vaug[t]
                nc.gpsimd.memset(va[:, :, D:D + 1], 1.0)
                acc = ps_po.tile([128, CC], F32, tag="pacc", name=f"vacc{t}")
                for k in range(KC):
                    nc.tensor.matmul(
                        acc[:], xt[k][:, t * 128:(t + 1) * 128], wv_s[k][:],
                        start=(k == 0), stop=(k == KC - 1))
                nc.vector.tensor_copy(
                    va[:, :, 0:D], acc[:].rearrange("p (h d) -> p h d", d=D))
            return f

        qk_acc = {}

        def mk_qk(which, m, blk, part=None):
            # part=0/1 emit half the contraction each (shorter PE bursts keep
            # the exp stream gapless); part=None emits the whole group
            ws, dst, bias = ((wq_s, qt, bq2) if which == "q"
                             else (wk_s, kt, bk2))

            def f():
                bc = slice(blk * BLK, (blk + 1) * BLK)
                if part in (None, 0):
                    qk_acc[(which, m, blk)] = ps_po.tile(
                        [128, BLK], F32, tag="pacc",
                        name=f"{which}acc{m}_{blk}")
                acc = qk_acc[(which, m, blk)]
                ks = range(KC) if part is None else \
                    range(part * KC // 2, (part + 1) * KC // 2)
                for k in ks:
                    nc.tensor.matmul(
                        acc[:], ws[k][:, m * 128:(m + 1) * 128], xt[k][:, bc],
                        start=(k == 0), stop=(k == KC - 1))
                if part in (None, 1):
                    nc.vector.tensor_scalar_add(
                        dst[m][:, bc], acc[:], bias[:, m:m + 1])
            return f

        def mk_op(t, half):
            def f():
                rows = slice(t * 128, (t + 1) * 128)
                cols = slice(half * BLK, (half + 1) * BLK)
                po = ps_po.tile([128, BLK], F32, tag="pacc",
                                name=f"po{t}_{half}")
                for m in range(MC):
                    nc.tensor.matmul(
                        po[:], ytn[m][:, rows], wp_s[m][:, cols],
                        start=(m == 0), stop=(m == MC - 1))
                st = p_st.tile([128, BLK], F32, tag="st", name=f"st{t}_{half}")
                nc.vector.tensor_copy(st[:], po[:])
                nc.sync.dma_start(out_d[rows, cols], st[:])
            return f

        # ---- filler schedule ----
        # NOTE: every block's av loop reads ALL 16 vaug chunks; the in-order
        # PE queue makes emission order the dependency order, so v4..15 sit
        # at the first 12 slots of (0,0). Pair m+1's q/k projections are
        # emitted as half-groups (4 matmuls) during pair m's blocks 1..3.
        filler = {}
        filler[(0, 0)] = [mk_v(4 + j) for j in range(12)]
        for m in (0, 1, 2):
            filler[(m, 1)] = [mk_qk("q", m + 1, b, pt)
                              for b, pt in ((0, 0), (0, 1), (1, 0), (1, 1),
                                            (2, 0), (2, 1))]
            filler[(m, 2)] = [mk_qk("q", m + 1, 3, 0), mk_qk("q", m + 1, 3, 1),
                              mk_qk("k", m + 1, 0, 0), mk_qk("k", m + 1, 0, 1),
                              mk_qk("k", m + 1, 1, 0), mk_qk("k", m + 1, 1, 1)]
            filler[(m, 3)] = [mk_qk("k", m + 1, 2, 0), mk_qk("k", m + 1, 2, 1),
                              mk_qk("k", m + 1, 3, 0), mk_qk("k", m + 1, 3, 1)]
        for blk in (1, 2, 3):
            filler[(3, blk)] = [mk_op(4 * (blk - 1) + j, h)
                                for j in range(4) for h in range(2)]

        # ---- prologue: pair-0 q/k projections + first quarter of v, with
        # each weight DMA emitted right before its first consumer group ----
        for blk in range(NB):
            mk_qk("q", 0, blk)()
        nc.sync.dma_start(wk_all[:], wk_d.rearrange("(c p) n -> p c n", p=128))
        for blk in range(NB):
            mk_qk("k", 0, blk)()
        nc.sync.dma_start(wv_all[:], wv_d.rearrange("(c p) n -> p c n", p=128))
        for t in range(4):
            mk_v(t)()
        nc.sync.dma_start(wp_all[:], wp_d.rearrange("(c p) n -> p c n", p=128))

        # ---- main pipelined attention loop: one flat stream of 256 chunks;
        # av lags scores/exp by 2 chunks and crosses block boundaries, so the
        # ACT exp stream never pauses at a block edge ----
        def normalize(m, blk, yaug):
            # row 64 of yaug is the softmax denominator; broadcast the raw
            # denominator first, then one fast (~51 ULP) reciprocal on all 64
            # partitions — keeps the DVE chain short so pair-3 outproj and
            # next-block av are not head-of-line blocked.
            bc = slice(blk * BLK, (blk + 1) * BLK)
            for h in range(2):
                ys = p_sm.tile([D + 1, BLK], F32, tag=f"ys{h}",
                               name=f"ys{m}_{blk}_{h}")
                nc.vector.tensor_copy(ys[:], yaug[h][:])
                rr = p_sm.tile([1, BLK], F32, tag=f"rr{h}",
                               name=f"rr{m}_{blk}_{h}")
                nc.vector.tensor_copy(rr[:], ys[D:D + 1, :])
                db = p_sm.tile([D, BLK], F32, tag=f"db{h}",
                               name=f"db{m}_{blk}_{h}")
                nc.gpsimd.partition_broadcast(db[:], rr[:])
                bb = p_sm.tile([D, BLK], F32, tag=f"bb{h}",
                               name=f"bb{m}_{blk}_{h}")
                nc.vector.reciprocal_approx_fast(bb[:], db[:])
                nc.vector.tensor_mul(
                    ytn[m][h * 64:h * 64 + 64, bc], ys[0:D, :], bb[:])

        def emit_av(gp):
            mp, gg = gp // (NB * TC), gp % TC
            bi = gp // TC
            if gg == 0:
                yaugs[bi] = [ps_y.tile([D + 1, BLK], F32, tag="yaug",
                                       name=f"yaug{bi}_{h}")
                             for h in range(2)]
            for h in range(2):
                nc.tensor.matmul(
                    yaugs[bi][h][:], vaug[gg][:, 2 * mp + h, :],
                    es[gp][:, h, :],
                    start=(gg == 0), stop=(gg == TC - 1))
            del es[gp]
            if gg == TC - 1:
                normalize(mp, (bi % NB), yaugs.pop(bi))

        NCH = MC * NB * TC  # 256 chunks total
        yaugs = {}  # block index -> psum tile pair
        es = {}     # global chunk index -> exp tile
        # 2-chunk cycles: both score pairs (64x128 tile mode) back-to-back,
        # then the lagged av pairs + fillers (128-wide mode) — halves the PE
        # tiling-mode switches per chunk.
        for gi2 in range(0, NCH + 2, 2):
            for gi in (gi2, gi2 + 1):
                if gi >= NCH:
                    continue
                m, blk, g = gi // (NB * TC), (gi // TC) % NB, gi % TC
                bc = slice(blk * BLK, (blk + 1) * BLK)
                tcols = slice(g * 128, (g + 1) * 128)
                s01 = ps_s.tile([128, 2, BLK], F32, tag="s01", name=f"s{gi}")
                for h in range(2):
                    hp = slice(h * 64, (h + 1) * 64)
                    nc.tensor.matmul(
                        s01[:, h, :], kt[m][hp, tcols], qt[m][hp, bc],
                        start=True, stop=True)
                e01 = p_e.tile([128, 2, BLK], BF16, tag="e", name=f"e{gi}")
                nc.scalar.activation(e01[:], s01[:], EXP, scale=0.125)
                es[gi] = e01
            for gp in (gi2 - 2, gi2 - 1):
                if 0 <= gp:
                    emit_av(gp)
            for gi in (gi2, gi2 + 1):
                if gi >= NCH:
                    continue
                m, blk, g = gi // (NB * TC), (gi // TC) % NB, gi % TC
                fills = filler.get((m, blk), [])
                if not fills:
                    continue
                if m == 3 and blk > 0:  # outproj: ytn[3] ready ~5 chunks in
                    slots = {5 + j: j for j in range(8)}
                else:  # evenly spaced, front-loaded (v JIT in (0,0))
                    n = len(fills)
                    slots = {(j * TC) // n: j for j in range(n)}
                if g in slots:
                    fills[slots[g]]()

        # ---- tail: output projection of the last block; stage all eight
        # half-chunks in one SBUF tile and ship a single DMA descriptor ----
        st_big = p_st.tile([128, 4, C], F32, tag="stb", bufs=1, name="st_big")
        for j in range(4):
            rows = slice((12 + j) * 128, (13 + j) * 128)
            for half in range(2):
                cols = slice(half * BLK, (half + 1) * BLK)
                po = ps_po.tile([128, BLK], F32, tag="pacc",
                                name=f"pot{j}_{half}")
                for m in range(MC):
                    nc.tensor.matmul(
                        po[:], ytn[m][:, rows], wp_s[m][:, cols],
                        start=(m == 0), stop=(m == MC - 1))
                nc.vector.tensor_copy(st_big[:, j, cols], po[:])
        nc.sync.dma_start(
            out_d[12 * 128:T, :].rearrange("(j p) c -> p j c", p=128),
            st_big[:])

    nc.compile()
    _nc_cache["nc"] = nc
    return nc


def prepare_in_maps(x, Wq, bq, Wk, bk, Wv, bv, Wp, bp):
    x = np.asarray(x, dtype=np.float32)
    Wq, bq = np.asarray(Wq, np.float32), np.asarray(bq, np.float32)
    Wk, bk = np.asarray(Wk, np.float32), np.asarray(bk, np.float32)
    Wv = np.asarray(Wv, np.float32)
    Wp = np.asarray(Wp, np.float32)
    bf = ml_dtypes.bfloat16

    in_maps = []
    for c in range(NCORES):
        b, half = divmod(c, 2)
        cols = slice(half * CC, (half + 1) * CC)
        in_maps.append({
            "xT": np.ascontiguousarray(x[b].T).astype(bf),
            "wq": np.ascontiguousarray(Wq[:, cols]).astype(bf),
            "wk": np.ascontiguousarray(Wk[:, cols]).astype(bf),
            "wv": np.ascontiguousarray(Wv[:, cols]).astype(bf),
            "wp": np.ascontiguousarray(Wp[cols, :]).astype(bf),
            "bq2": np.ascontiguousarray(bq[cols].reshape(4, 128).T),
            "bk2": np.ascontiguousarray(bk[cols].reshape(4, 128).T),
        })
    return in_maps


def combine(results, Wv, bv, Wp, bp):
    bv = np.asarray(bv, np.float32)
    Wp = np.asarray(Wp, np.float32)
    bp = np.asarray(bp, np.float32)
    out = np.zeros((B, T, C), np.float32)
    for c in range(NCORES):
        b, half = divmod(c, 2)
        cols = slice(half * CC, (half + 1) * CC)
        out[b] += results[c]["out"]
        # bv enters y as att@1 * bv = bv per row (softmax rows sum to 1)
        out[b] += bv[cols] @ Wp[cols, :]
    out += bp
    return out


def kernel(x, Wq, bq, Wk, bk, Wv, bv, Wp, bp):
    in_maps = prepare_in_maps(x, Wq, bq, Wk, bk, Wv, bv, Wp, bp)
    nc = _build_nc()
    res = run_bass_kernel_spmd(nc, in_maps, list(range(NCORES))).results
    return combine(res, Wv, bv, Wp, bp)
